# revision 6
# baseline (speedup 1.0000x reference)
"""DeepseekV3 mini MoE MLP on 8 TRN2 NeuronCores.

Strategy: expert-parallel. The router (tiny: 0.1% of FLOPs) is computed
with jax ops that mirror the reference bit-for-bit; tokens are then
dispatched on the host to per-expert batches (the "all-to-all"), one
expert per NeuronCore. Each core runs a fused gate/up/silu/mul/down
kernel over its routed tokens. The combine (scatter-add weighted by the
top-k routing weights) happens on the host.

Matmul operands are fp16 (PE upconverts to FP22 internally, same
throughput as f32r but half the DMA traffic and SBUF footprint, and
fast-weight-load applies). PSUM accumulation stays fp32.

Layouts are feature-major ([dim, tokens]) so every matmul contracts over
the SBUF partition dim with no transposes anywhere on device. Weights
are packed on the host as [k_in(P), m_blk, k_blk, m_in] so one matrix is
1-2 fully-contiguous DMAs: a small "starter" slab (output blocks 0-1)
that lands within ~3us so the PE can start, and the bulk slab (blocks
2-7). gate/up slabs ride the Activation HWDGE ring; x tiles, the up bulk
slab and down weights ride the SP ring — no ring ever makes the PE wait.
"""

import numpy as np

import concourse.bass as bass
import concourse.mybir as mybir
import concourse.tile as tile
from concourse import bacc
from concourse.bass_utils import run_bass_kernel_spmd

DIM = 1024
HIDDEN = 1024
NUM_EXPERTS = 8
TOP_K = 2
P = 128
TT = 512  # main token tile (PSUM bank = 512 fp32)
DT = DIM // P  # 8 d-tiles
HT = HIDDEN // P  # 8 h-tiles
SW = 2  # h-blocks covered by the starter weight slab

F32 = mybir.dt.float32
F16 = mybir.dt.float16

_program_cache: dict[tuple, object] = {}
LAST_RESULT = None


def _build_expert_program(tiles: tuple):
    """One-expert FFN: yt = ((silu(x@wg.T) * (x@wu.T)) @ wd.T).T over C tokens.

    DRAM params (per core), all fp16:
      xt [DIM, C]          tokens, transposed (d-major)
      wg/wu [P, HT*DT*P]   gate/up proj packed [k_in, m_blk, k_blk, m_in]
      wd [P, HT*HT*P]      down proj packed    [k_in, m_blk, k_blk, m_in]
      yt [DIM, C]          output, transposed
    """
    C = sum(tiles)
    nc = bacc.Bacc(None, target_bir_lowering=False, debug=False)
    xt = nc.declare_dram_parameter("xt", [DIM, C], F16, isOutput=False)
    wg = nc.declare_dram_parameter("wg", [P, HT * DT * P], F16, isOutput=False)
    wu = nc.declare_dram_parameter("wu", [P, HT * DT * P], F16, isOutput=False)
    wd = nc.declare_dram_parameter("wd", [P, HT * HT * P], F16, isOutput=False)
    yt = nc.declare_dram_parameter("yt", [DIM, C], F16, isOutput=True)

    # gate/up weight slabs: h-blocks [0,1), [1,3), [3,5), [5,8) — sized so
    # each lands (at ~230GB/s on the Activation ring) before the PE needs it.
    WCUTS = (0, 1, 3, 5, HT)

    with tile.TileContext(nc) as tc:
        with (
            tc.tile_pool(name="wpool", bufs=1) as wpool,
            tc.tile_pool(name="xpool", bufs=3) as xpool,
            tc.tile_pool(name="hpool", bufs=2) as hpool,
            tc.tile_pool(name="apool", bufs=3) as apool,
            tc.tile_pool(name="ypool", bufs=3) as ypool,
            tc.tile_pool(name="pg", bufs=2, space="PSUM") as pgpool,
            tc.tile_pool(name="pu", bufs=2, space="PSUM") as pupool,
            tc.tile_pool(name="py", bufs=2, space="PSUM") as pypool,
        ):
            wg_t, wu_t = [], []
            for i in range(len(WCUTS) - 1):
                n = (WCUTS[i + 1] - WCUTS[i]) * DT * P
                wg_t.append(wpool.tile([P, n], F16, name=f"wg{i}", tag=f"wg{i}"))
                wu_t.append(wpool.tile([P, n], F16, name=f"wu{i}", tag=f"wu{i}"))
            wd_a = wpool.tile([P, HT * HT * P], F16, name="wd_a", tag="wd_a")

            def w_at(tiles_, h, a):
                i = next(j for j in range(len(WCUTS) - 1) if h < WCUTS[j + 1])
                off = ((h - WCUTS[i]) * DT + a) * P
                return tiles_[i][:, off : off + P]

            # PE warm-up: a few matmuls on zeroed SBUF start the HAM busy
            # window while the first x/weight DMAs are still in flight.
            wz = wpool.tile([P, P], F16, name="wz", tag="wz")
            xz = wpool.tile([P, TT], F16, name="xz", tag="xz")
            nc.vector.memset(wz[:, :], 0.0)
            nc.vector.memset(xz[:, :], 0.0)
            pz = pgpool.tile([P, TT], F32, tag="pg")
            for _ in range(6):
                nc.tensor.matmul(pz[:, :], wz[:, :], xz[:, :], start=True, stop=True)

            first = True
            off = 0
            XH = DT // 2  # d-blocks per x half-tile
            for t, tt in enumerate(tiles):
                ts = bass.ds(off, tt)
                off += tt
                # two half-tiles per token tile: the PE can start a tile's
                # matmuls when the first 4 d-blocks have landed.
                x_h = [
                    xpool.tile([P, XH * TT], F16, name=f"x{i}_{t}", tag=f"x{i}")
                    for i in range(2)
                ]
                for i in range(2):
                    nc.sync.dma_start(
                        out=x_h[i][:, :].rearrange("p (a t) -> p a t", a=XH)[:, :, :tt],
                        in_=xt.ap()[i * (DIM // 2) : (i + 1) * (DIM // 2), ts].rearrange(
                            "(a p) t -> p a t", p=P
                        ),
                    )

                def x_at(a):
                    return x_h[a // XH][:, (a % XH) * TT : (a % XH) * TT + tt]

                if first:
                    # Weight slabs split across both HWDGE rings, behind each
                    # ring's first obligation (x0 on SP), in consumption
                    # order: gate slabs ride SP, up slabs + down ride
                    # Activation. Each ring streams ~2-4MB while tile 0's
                    # (mostly cold-clock) matmuls run.
                    for i in range(len(WCUTS) - 1):
                        lo, hi = WCUTS[i] * DT * P, WCUTS[i + 1] * DT * P
                        nc.sync.dma_start(out=wg_t[i][:, :], in_=wg.ap()[:, lo:hi])
                        nc.scalar.dma_start(out=wu_t[i][:, :], in_=wu.ap()[:, lo:hi])
                    nc.scalar.dma_start(out=wd_a[:, :], in_=wd.ap()[:, :])
                    first = False

                h_sb = hpool.tile([P, HT * TT], F16, tag="h")
                for h in range(HT):
                    pg = pgpool.tile([P, tt], F32, tag="pg")
                    pu = pupool.tile([P, tt], F32, tag="pu")
                    for a in range(DT):
                        nc.tensor.matmul(
                            pg[:, :],
                            w_at(wg_t, h, a),
                            x_at(a),
                            start=(a == 0),
                            stop=(a == DT - 1),
                        )
                    for a in range(DT):
                        nc.tensor.matmul(
                            pu[:, :],
                            w_at(wu_t, h, a),
                            x_at(a),
                            start=(a == 0),
                            stop=(a == DT - 1),
                        )
                    act_sb = apool.tile([P, TT], F32, tag="act")
                    nc.scalar.activation(
                        act_sb[:, :tt], pg[:, :], mybir.ActivationFunctionType.Silu
                    )
                    nc.vector.tensor_tensor(
                        h_sb[:, h * TT : h * TT + tt],
                        act_sb[:, :tt],
                        pu[:, :],
                        mybir.AluOpType.mult,
                    )

                for do in range(HT):
                    py = pypool.tile([P, tt], F32, tag="py")
                    for a in range(HT):
                        nc.tensor.matmul(
                            py[:, :],
                            wd_a[:, (do * HT + a) * P : (do * HT + a) * P + P],
                            h_sb[:, a * TT : a * TT + tt],
                            start=(a == 0),
                            stop=(a == HT - 1),
                        )
                    y_sb = ypool.tile([P, TT], F16, tag="y")
                    nc.vector.tensor_copy(y_sb[:, :tt], py[:, :])
                    nc.scalar.dma_start(
                        out=yt.ap()[do * P : (do + 1) * P, ts], in_=y_sb[:, :tt]
                    )
    nc.compile()
    return nc


def _tiles_for(max_cnt: int) -> tuple:
    """Token tiles covering max_cnt: a 384-token ramp tile (small so the
    first x DMA lands fast), full 512s, and a >=256 multiple-of-64 tail."""
    if max_cnt <= 384:
        return (max(256, ((max_cnt + 63) // 64) * 64),)
    rem = max_cnt - 384
    full, tail = divmod(rem, TT)
    tail = ((tail + 63) // 64) * 64
    if tail == 0:
        return (384,) + (TT,) * full
    if tail < 256:
        if full == 0:
            return (384, max(256, tail))
        full -= 1
        tail += TT  # in (512, 768): split into two >=256 tiles
        a = 256
        return (384,) + (TT,) * full + (a, tail - a)
    return (384,) + (TT,) * full + (tail,)


def _get_program(tiles: tuple):
    if tiles not in _program_cache:
        _program_cache[tiles] = _build_expert_program(tiles)
    return _program_cache[tiles]


def _pack_w(wt: np.ndarray) -> np.ndarray:
    """[K, M] weight (K contracted) -> [k_in(P), m_blk, k_blk, m_in] flattened
    to [P, M//P * K//P * P]; slab for output block mb is contiguous per row."""
    K, M = wt.shape
    return np.ascontiguousarray(
        wt.astype(np.float16)
        .reshape(K // P, P, M // P, P)
        .transpose(1, 2, 0, 3)
        .reshape(P, (M // P) * (K // P) * P)
    )


def _route(flat: np.ndarray, gate_w: np.ndarray):
    """Mirror the reference router bit-for-bit (jax ops, same backend)."""
    try:
        import jax
        import jax.numpy as jnp

        logits = jnp.asarray(flat) @ jnp.asarray(gate_w).T
        scores = jax.nn.sigmoid(logits)
        top_val, top_idx = jax.lax.top_k(scores, TOP_K)
        top_val = top_val / (top_val.sum(-1, keepdims=True) + 1e-9)
        return np.asarray(top_val), np.asarray(top_idx)
    except Exception:
        # numpy fallback: identical selection semantics (stable descending)
        logits = flat @ gate_w.T
        scores = 1.0 / (1.0 + np.exp(-logits))
        order = np.argsort(-scores, axis=-1, kind="stable")
        top_idx = order[:, :TOP_K].astype(np.int32)
        top_val = np.take_along_axis(scores, top_idx, axis=-1)
        top_val = top_val / (top_val.sum(-1, keepdims=True) + 1e-9)
        return top_val.astype(np.float32), top_idx


def kernel(x, gate_w, gate_proj, up_proj, down_proj):
    x = np.asarray(x)
    bsz, seqlen, dim = x.shape
    flat = np.ascontiguousarray(x.reshape(-1, dim), dtype=np.float32)
    T = flat.shape[0]
    gate_w = np.asarray(gate_w, dtype=np.float32)
    gate_proj = np.asarray(gate_proj, dtype=np.float32)
    up_proj = np.asarray(up_proj, dtype=np.float32)
    down_proj = np.asarray(down_proj, dtype=np.float32)

    top_val, top_idx = _route(flat, gate_w)

    idx_list = []
    cw_list = []
    for e in range(NUM_EXPERTS):
        mask = top_idx == e  # [T, K]
        tok = np.nonzero(mask.any(axis=1))[0]
        w = (top_val * mask).sum(axis=1)[tok].astype(np.float32)
        idx_list.append(tok)
        cw_list.append(w)

    max_cnt = max(len(i) for i in idx_list)
    tiles = _tiles_for(max_cnt)
    C = sum(tiles)
    nc = _get_program(tiles)

    flat16 = flat.astype(np.float16)
    in_maps = []
    for e in range(NUM_EXPERTS):
        tok = idx_list[e]
        cnt = len(tok)
        xt = np.zeros((DIM, C), dtype=np.float16)
        xt[:, :cnt] = flat16[tok].T
        in_maps.append(
            {
                "xt": xt,
                "wg": _pack_w(gate_proj[e].T),
                "wu": _pack_w(up_proj[e].T),
                "wd": _pack_w(down_proj[e].T),
            }
        )

    res = run_bass_kernel_spmd(nc, in_maps, core_ids=list(range(NUM_EXPERTS)))
    global LAST_RESULT
    LAST_RESULT = res

    out = np.zeros((T, DIM), dtype=np.float32)
    for e in range(NUM_EXPERTS):
        tok = idx_list[e]
        cnt = len(tok)
        if cnt:
            yt = np.asarray(res.results[e]["yt"][:, :cnt], dtype=np.float32)
            out[tok] += (yt * cw_list[e][None, :]).T
    return out.reshape(bsz, seqlen, dim)
